# revision 39
# baseline (speedup 1.0000x reference)
"""Trainium2 Bass kernel for nn_Equi_Nonlin_Grad_Module (e3nn FCTP l⊗l→0e + MLP, fwd + input-grad).

Strategy (8 cores): K-shard the 21504-row (u,v)-pair contraction dim of the
fused tensor-product weight Wcat across cores (by u-blocks within each irrep
path: 16/8/4 u's per core -> 2688 rows each).  Each core computes its s-slice
for all N on DVE, PE-transposes it, does the partial forward matmul in fp32r,
ReduceScatters z (rank c receives its 128-row n-shard), runs the MLP fwd+bwd
on its shard, AllGathers dz, computes G[:, slice] = dz @ Wslice^T (PE-transposed
W), and finally the per-node gradient contributions on DVE/GPSIMD.  Partial
input-gradients are summed on the host.
"""

import sys

for _p in ("/opt/trn_rl_repo",):
    if _p not in sys.path:
        sys.path.insert(0, _p)

import numpy as np
import concourse.bacc as bacc
import concourse.mybir as mybir
import concourse.tile as tile
from concourse import masks
from concourse.bass_utils import run_bass_kernel_spmd

F32 = mybir.dt.float32
F32R = mybir.dt.float32r
AOP = mybir.AluOpType
AFT = mybir.ActivationFunctionType
AXT = mybir.AxisListType

NCORES = 8
N = 1024
NT = 8  # n-tiles of 128
DIM = 480
HID = 1024
ZD = 256
MULS = ((128, 1), (64, 3), (32, 5))
PATH_OFF = (0, 128, 320)  # column offsets of each irrep path in x
ALPHA = 1.0 / np.sqrt(float(128 * 128 + 64 * 64 + 32 * 32))
# per-core u-counts per path
UC = (16, 8, 4)
NKG = 21  # 16 path0 + 4 path1 + 1 path2 pair-row tiles of 128
KROWS = NKG * 128  # 2688
NCH = 7  # backward chunks of 3 kg = 384 cols
EPS = 1e-6


def _kg_info(kg):
    """(path, list of (u_local, mul, d, s_col0, width)) for pair-tile kg."""
    if kg < 16:
        return 0, [(kg, 128, 1, 0, 128)]
    if kg < 20:
        j = kg - 16
        return 1, [(2 * j, 64, 3, 0, 64), (2 * j + 1, 64, 3, 64, 64)]
    return 2, [(q, 32, 5, q * 32, 32) for q in range(4)]


def _ucol_index(path, u_local, m):
    """Column in the per-core xu / dxu [*, 60] tensor."""
    if path == 0:
        return u_local
    if path == 1:
        return 16 + u_local * 3 + m
    return 40 + u_local * 5 + m


def host_ucols(core):
    """Global x-column index for each of the 60 per-core u-scalar columns."""
    cols = []
    for u in range(16 * core, 16 * core + 16):
        cols.append(0 + u)  # path0, d=1
    for u in range(8 * core, 8 * core + 8):
        for m in range(3):
            cols.append(128 + u * 3 + m)
    for u in range(4 * core, 4 * core + 4):
        for m in range(5):
            cols.append(320 + u * 5 + m)
    return np.array(cols, dtype=np.int64)


def host_wslice(W0, W1, W2, core):
    """Per-core [2688, 1024] slice of the pre-scaled fused weight."""
    w0 = W0.reshape(128 * 128, HID)
    w1 = W1.reshape(64 * 64, HID)
    w2 = W2.reshape(32 * 32, HID)
    rows = []
    for u in range(16 * core, 16 * core + 16):
        rows.append(w0[u * 128 : (u + 1) * 128] * (ALPHA / np.sqrt(1.0)))
    for j in range(4):
        for h in range(2):
            u = 8 * core + 2 * j + h
            rows.append(w1[u * 64 : (u + 1) * 64] * (ALPHA / np.sqrt(3.0)))
    for q in range(4):
        u = 4 * core + q
        rows.append(w2[u * 32 : (u + 1) * 32] * (ALPHA / np.sqrt(5.0)))
    return np.ascontiguousarray(np.concatenate(rows, 0), dtype=np.float32)


def _x_path_view(ap, nt_unused, path, m, mul):
    """[128, mul] strided view of the (v, m) columns of path in an x-tile AP."""
    off, d = PATH_OFF[path], MULS[path][1]
    if d == 1:
        return ap[:, off : off + mul]
    v = ap[:, off : off + mul * d].rearrange("p (v m) -> p m v", m=d)
    return v[:, m, :]


def build_kernel():
    import os

    PH = int(os.environ.get("KPHASE", "9"))  # 1=fwd-TP+RS, 2=+MLP, 3=+AG+bwd, 9=all
    KSTOP = int(os.environ.get("KSTOP", "99"))  # 20=L1, 21=fwd, 22=dh3, 99=all
    KTIME = int(os.environ.get("KTIME", "0"))  # 1: replace collectives with local DMA (for TimelineSim)
    nc = bacc.Bacc("TRN2", target_bir_lowering=False, debug=False, num_devices=NCORES)

    # ---------------- I/O ----------------
    x_in = nc.dram_tensor("x_in", [N, DIM], F32, kind="ExternalInput")
    xu_in = nc.dram_tensor("xu_in", [N, 60], F32, kind="ExternalInput")
    w_in = nc.dram_tensor("w_in", [KROWS, HID], F32, kind="ExternalInput")
    # fp32r alias of the same weight bytes: lets HWDGE load f32r tiles with no cast
    wr_in = nc.dram_tensor("wr_in", [KROWS, HID], F32R, kind="ExternalInput")
    w1_in = nc.dram_tensor("w1_in", [HID, ZD], F32, kind="ExternalInput")
    w2_in = nc.dram_tensor("w2_in", [ZD, ZD], F32, kind="ExternalInput")
    w3_in = nc.dram_tensor("w3_in", [ZD, ZD], F32, kind="ExternalInput")
    w4_in = nc.dram_tensor("w4_in", [ZD, ZD], F32, kind="ExternalInput")
    w1r_in = nc.dram_tensor("w1r_in", [HID, ZD], F32R, kind="ExternalInput")
    w2r_in = nc.dram_tensor("w2r_in", [ZD, ZD], F32R, kind="ExternalInput")
    w3r_in = nc.dram_tensor("w3r_in", [ZD, ZD], F32R, kind="ExternalInput")
    w4r_in = nc.dram_tensor("w4r_in", [ZD, ZD], F32R, kind="ExternalInput")
    vecs_in = nc.dram_tensor("vecs_in", [10 * 128, ZD], F32, kind="ExternalInput")
    xout = nc.dram_tensor("xout", [128, ZD], F32, kind="ExternalOutput")
    dxu_out = nc.dram_tensor("dxu_out", [N, 60], F32, kind="ExternalOutput")
    dx2_out = nc.dram_tensor("dx2_out", [N, DIM], F32, kind="ExternalOutput")

    rg = [list(range(NCORES))]
    VEC = {"b1": 0, "g1": 1, "be1": 2, "b2": 3, "g2": 4, "be2": 5, "b3": 6, "g3": 7, "be3": 8, "b4": 9}

    with tile.TileContext(nc) as tc:
        with (
            tc.tile_pool(name="const", bufs=1) as cp,
            tc.tile_pool(name="psum", bufs=1, space="PSUM") as pp,
            tc.tile_pool(name="dram", bufs=1, space="DRAM") as dram,
            tc.tile_pool(name="work", bufs=1) as wk,
        ):
            # ---------------- phase A: constants + persistent loads ----------------
            ident = cp.tile([128, 128], F32, name="ident")
            masks.make_identity(nc, ident[:])
            ones_f = cp.tile([128, 128], F32, name="ones_f")
            nc.gpsimd.memset(ones_f[:], 1.0)
            ones_r = cp.tile([128, 128], F32R, name="ones_r")
            nc.vector.tensor_copy(ones_r[:], ones_f[:])
            eps_t = cp.tile([128, 1], F32, name="eps_t")
            nc.gpsimd.memset(eps_t[:], EPS)

            xt = []
            xut = []
            dx2t = []
            dxut = []
            for nt in range(NT):
                t = cp.tile([128, DIM], F32, name=f"xt{nt}")
                nc.sync.dma_start(t[:], x_in[nt * 128 : (nt + 1) * 128, :])
                xt.append(t)
                t = cp.tile([128, 60], F32, name=f"xut{nt}")
                nc.sync.dma_start(t[:], xu_in[nt * 128 : (nt + 1) * 128, :])
                xut.append(t)
                t = cp.tile([128, DIM], F32, name=f"dx2t{nt}")
                nc.gpsimd.memset(t[:], 0.0)
                dx2t.append(t)
                t = cp.tile([128, 60], F32, name=f"dxut{nt}")
                dxut.append(t)

            # LN/bias vectors arrive pre-broadcast from the host: [10*128, ZD]
            vb = {}
            for k, i in VEC.items():
                t = cp.tile([128, ZD], F32, name=f"vb_{k}")
                nc.sync.dma_start(t[:], vecs_in[i * 128 : (i + 1) * 128, :])
                vb[k] = t

            # MLP weights: natural fp32r (fwd) and fp32 (for transposing)
            w1t, w1f = [], []
            for k in range(8):
                t = wk.tile([128, ZD], F32R, name=f"w1t{k}")
                nc.sync.dma_start(t[:], w1r_in[k * 128 : (k + 1) * 128, :])
                w1t.append(t)
                t = wk.tile([128, ZD], F32, name=f"w1f{k}")
                nc.sync.dma_start(t[:], w1_in[k * 128 : (k + 1) * 128, :])
                w1f.append(t)
            wnt = {}
            wnf = {}
            for nm, dtr, dt_ in (("w2", w2r_in, w2_in), ("w3", w3r_in, w3_in), ("w4", w4r_in, w4_in)):
                wnt[nm], wnf[nm] = [], []
                for k in range(2):
                    t = wk.tile([128, ZD], F32R, name=f"{nm}t{k}")
                    nc.sync.dma_start(t[:], dtr[k * 128 : (k + 1) * 128, :])
                    wnt[nm].append(t)
                    t = wk.tile([128, ZD], F32, name=f"{nm}f{k}")
                    nc.sync.dma_start(t[:], dt_[k * 128 : (k + 1) * 128, :])
                    wnf[nm].append(t)

            # DRAM bounce buffers for collectives
            z_part_b = dram.tile([N, HID], F32, name="z_part_b")
            z_rs_b = dram.tile([128, HID], F32, name="z_rs_b")
            dz_in_b = dram.tile([128, HID], F32, name="dz_in_b")
            dz_ag_b = dram.tile(
                [N, HID], F32, name="dz_ag_b", addr_space=("Local" if KTIME else "Shared")
            )

            def evac(dst, src, idx):
                # PSUM->SBUF evacuation of small transpose tiles: keep on ACT
                # (DVE is the bottleneck engine; ACT has headroom)
                if idx % 4 == 3:
                    nc.vector.tensor_copy(dst, src)
                else:
                    nc.scalar.activation(dst, src, AFT.Copy)

            # ---------------- phase B: forward tensor-product ----------------
            with (
                tc.tile_pool(name="wpool", bufs=1) as wp,
                tc.tile_pool(name="fwd", bufs=3) as fp,
            ):
                wt = []
                for kg in range(NKG):
                    t = wp.tile([128, HID], F32R, name=f"wt{kg}")
                    nc.sync.dma_start(t[:], wr_in[kg * 128 : (kg + 1) * 128, :])
                    wt.append(t)

                ei = 0
                for nt in range(NT):
                    zp0 = pp.tile([128, 512], F32, tag="zpA", bufs=2, name="zp0")
                    zp1 = pp.tile([128, 512], F32, tag="zpB", bufs=2, name="zp1")
                    # 1) build all 21 transposed s tiles for this n-tile
                    sTs = []
                    for kg in range(NKG):
                        path, groups = _kg_info(kg)
                        d = MULS[path][1]
                        s_blk = fp.tile([128, 128], F32, tag="s_blk", bufs=4, name="s_blk")
                        eng = nc.vector
                        for (ul, mul, _d, c0, w) in groups:
                            sv = s_blk[:, c0 : c0 + w]
                            for m in range(d):
                                uc = xut[nt][:, _ucol_index(path, ul, m) : _ucol_index(path, ul, m) + 1]
                                xv = _x_path_view(xt[nt][:], nt, path, m, mul)
                                if m == 0:
                                    # tensor_scalar_mul on ACT (Copy w/ per-partition scale)
                                    nc.scalar.activation(sv, xv, AFT.Copy, scale=uc)
                                else:
                                    eng.scalar_tensor_tensor(
                                        sv, xv, uc, sv, op0=AOP.mult, op1=AOP.add
                                    )
                        tp = pp.tile([128, 128], F32, tag="tp", bufs=4, name="tp")
                        nc.tensor.transpose(tp[:], s_blk[:], ident[:])
                        sT = fp.tile([128, 128], F32R, tag=f"sT{kg}", bufs=2, name="sT")
                        evac(sT[:], tp[:], ei)
                        ei += 1
                        sTs.append(sT)
                    # 2) dense back-to-back matmul accumulation
                    for kg in range(NKG):
                        nc.tensor.matmul(
                            zp0[:], sTs[kg][:], wt[kg][:, 0:512], start=(kg == 0), stop=(kg == NKG - 1)
                        )
                        nc.tensor.matmul(
                            zp1[:], sTs[kg][:], wt[kg][:, 512:HID], start=(kg == 0), stop=(kg == NKG - 1)
                        )
                    z_sb = fp.tile([128, HID], F32, tag="z_sb", bufs=2, name="z_sb")
                    nc.vector.tensor_copy(z_sb[:, 0:512], zp0[:])
                    nc.vector.tensor_copy(z_sb[:, 512:HID], zp1[:])
                    nc.sync.dma_start(z_part_b[nt * 128 : (nt + 1) * 128, :], z_sb[:])

            # ---------------- phase C: ReduceScatter z ----------------
            if KTIME:
                nc.sync.dma_start(z_rs_b[:], z_part_b[0:128, :])
            else:
                nc.gpsimd.collective_compute(
                    "ReduceScatter", AOP.add, replica_groups=rg,
                    ins=[z_part_b[:].opt()], outs=[z_rs_b[:].opt()],
                )

            if PH == 1:
                zdbg = wk.tile([128, HID], F32, name="zdbg")
                nc.sync.dma_start(zdbg[:], z_rs_b[:])
                nc.sync.dma_start(xout[:], zdbg[:, 0:ZD])

            # ---------------- phase D/E: MLP fwd + bwd on the local n-shard ----------------
            if PH >= 2:
              with (
                tc.tile_pool(name="mlp", bufs=1) as mp,
                tc.tile_pool(name="mps", bufs=1, space="PSUM") as mps,
            ):
                z_loc = mp.tile([128, HID], F32, name="z_loc")
                nc.sync.dma_start(z_loc[:], z_rs_b[:])

                ti = 0

                def pe_T(dst_f32r_ap, src_f32_ap):
                    nonlocal ti
                    tp = pp.tile([128, 128], F32, tag="tp", bufs=4, name="mtp")
                    nc.tensor.transpose(tp[:], src_f32_ap, ident[:])
                    evac(dst_f32r_ap, tp[:], ti)
                    ti += 1

                zT = []
                for k in range(8):
                    t = mp.tile([128, 128], F32R, name=f"zT{k}")
                    pe_T(t[:], z_loc[:, k * 128 : (k + 1) * 128])
                    zT.append(t)

                def ln_silu(a_ps, lidx, apply_silu=True):
                    """Input: psum [128, ZD] pre-bias. Returns (h, xh, inv, t_ln)."""
                    s = str(lidx)
                    a = mp.tile([128, ZD], F32, name=f"a{s}")
                    nc.vector.tensor_add(a[:], a_ps[:], vb["b" + s][:])
                    if KSTOP == 10 and lidx == 1:
                        nc.sync.dma_start(xout[:], a[:])
                        return a, a, a, a
                    red = mp.tile([128, 1], F32, name=f"red{s}")
                    nc.vector.tensor_reduce(red[:], a[:], AXT.X, AOP.add)
                    negmean = mp.tile([128, 1], F32, name=f"negmean{s}")
                    nc.scalar.activation(negmean[:], red[:], AFT.Copy, scale=-1.0 / ZD)
                    xc = mp.tile([128, ZD], F32, name=f"xc{s}")
                    nc.vector.tensor_scalar_add(xc[:], a[:], negmean[:])
                    if KSTOP == 11 and lidx == 1:
                        nc.sync.dma_start(xout[:], xc[:])
                        return a, a, a, a
                    junk = mp.tile([128, ZD], F32, tag="mjunk", bufs=2, name="mjunk")
                    vsum = mp.tile([128, 1], F32, name=f"vsum{s}")
                    nc.vector.tensor_mul(junk[:], xc[:], xc[:])
                    nc.vector.tensor_reduce(vsum[:], junk[:], AXT.X, AOP.add)
                    if KSTOP == 13 and lidx == 1:
                        nc.sync.dma_start(xout[:], junk[:])
                        return a, a, a, a
                    sig = mp.tile([128, 1], F32, name=f"sig{s}")
                    nc.scalar.activation(sig[:], vsum[:], AFT.Sqrt, scale=1.0 / ZD, bias=eps_t[:])
                    inv = mp.tile([128, 1], F32, name=f"inv{s}")
                    if KSTOP == 14 and lidx == 1:
                        nc.sync.dma_start(xout[:], junk[:])
                        return a, a, a, a
                    nc.vector.reciprocal(inv[:], sig[:])
                    if KSTOP == 12 and lidx == 1:
                        nc.sync.dma_start(xout[:], junk[:])
                        return a, a, a, a
                    xh = mp.tile([128, ZD], F32, name=f"xh{s}")
                    nc.vector.tensor_scalar_mul(xh[:], xc[:], inv[:])
                    t_ln = mp.tile([128, ZD], F32, name=f"tln{s}")
                    nc.vector.tensor_mul(t_ln[:], xh[:], vb["g" + s][:])
                    nc.vector.tensor_add(t_ln[:], t_ln[:], vb["be" + s][:])
                    sg = mp.tile([128, ZD], F32, name=f"sg{s}")
                    nc.scalar.activation(sg[:], t_ln[:], AFT.Sigmoid)
                    h = mp.tile([128, ZD], F32, name=f"h{s}")
                    nc.vector.tensor_mul(h[:], t_ln[:], sg[:])
                    return h, xh, inv, sg

                # layer 1
                ap1 = pp.tile([128, ZD], F32, tag="zpA", bufs=2, name="ap1")
                for k in range(8):
                    nc.tensor.matmul(ap1[:], zT[k][:], w1t[k][:], start=(k == 0), stop=(k == 7))
                h1, xh1, inv1, sg1 = ln_silu(ap1, 1)
                if KSTOP == 20:
                    nc.sync.dma_start(xout[:], h1[:])

                def fwd_layer(h_prev, wts, lidx):
                    hT = []
                    for k in range(2):
                        t = mp.tile([128, 128], F32R, name=f"hT{lidx}_{k}")
                        pe_T(t[:], h_prev[:, k * 128 : (k + 1) * 128])
                        hT.append(t)
                    aps = pp.tile([128, ZD], F32, tag="zpA", bufs=2, name=f"aps{lidx}")
                    for k in range(2):
                        nc.tensor.matmul(aps[:], hT[k][:], wts[k][:], start=(k == 0), stop=(k == 1))
                    return aps

                if KSTOP > 20:
                  ap2 = fwd_layer(h1, wnt["w2"], 2)
                  h2, xh2, inv2, sg2 = ln_silu(ap2, 2)
                  ap3 = fwd_layer(h2, wnt["w3"], 3)
                  h3, xh3, inv3, sg3 = ln_silu(ap3, 3)
                  ap4 = fwd_layer(h3, wnt["w4"], 4)
                  xo = mp.tile([128, ZD], F32, name="xo")
                  nc.vector.tensor_add(xo[:], ap4[:], vb["b4"][:])
                  nc.sync.dma_start(xout[:], xo[:])

                # ---- MLP backward ----
                def transpose_w(wf_tiles, nm, wide):
                    """build w^T fp32r tiles: wide=ZD cols -> [2][128, 128*len(wf)]"""
                    ncols = 128 * len(wf_tiles)
                    out = []
                    for k in range(wide // 128):
                        t = mp.tile([128, ncols], F32R, name=f"{nm}T{k}")
                        out.append(t)
                    for jt, wf in enumerate(wf_tiles):
                        for k in range(wide // 128):
                            pe_T(out[k][:, jt * 128 : (jt + 1) * 128], wf[:, k * 128 : (k + 1) * 128])
                    return out

                if KSTOP > 21:
                  w4T = transpose_w(wnf["w4"], "w4", ZD)
                  dh3ps = pp.tile([128, ZD], F32, tag="zpB", bufs=2, name="dh3ps")
                  for k in range(2):
                    nc.tensor.matmul(dh3ps[:], ones_r[:], w4T[k][:], start=(k == 0), stop=(k == 1))
                  dh3 = mp.tile([128, ZD], F32, name="dh3")
                  nc.vector.tensor_copy(dh3[:], dh3ps[:])

                def bwd_layer(dh, h, sg, xh, inv, lidx):
                    """returns da = dL/d(pre-LN activations) [128, ZD] fp32 sbuf"""
                    s = str(lidx)
                    # silu'(t) = sg + h*(1 - sg)  (h = t*sg)
                    ds = mp.tile([128, ZD], F32, tag="mjunk2", bufs=2, name="ds")
                    nc.vector.tensor_mul(ds[:], h[:], sg[:])
                    nc.vector.tensor_sub(ds[:], h[:], ds[:])
                    nc.vector.tensor_add(ds[:], ds[:], sg[:])
                    dt = mp.tile([128, ZD], F32, name=f"dt{s}")
                    nc.vector.tensor_mul(dt[:], dh[:], ds[:])
                    dxh = mp.tile([128, ZD], F32, name=f"dxh{s}")
                    nc.vector.tensor_mul(dxh[:], dt[:], vb["g" + s][:])
                    m1r = mp.tile([128, 1], F32, name=f"m1r{s}")
                    nc.vector.tensor_reduce(m1r[:], dxh[:], AXT.X, AOP.add)
                    negm1 = mp.tile([128, 1], F32, name=f"negm1{s}")
                    nc.scalar.activation(negm1[:], m1r[:], AFT.Copy, scale=-1.0 / ZD)
                    junk = mp.tile([128, ZD], F32, tag="mjunk", bufs=2, name="mjunkb")
                    m2r = mp.tile([128, 1], F32, name=f"m2r{s}")
                    nc.vector.tensor_mul(junk[:], dxh[:], xh[:])
                    nc.vector.tensor_reduce(m2r[:], junk[:], AXT.X, AOP.add)
                    negm2 = mp.tile([128, 1], F32, name=f"negm2{s}")
                    nc.scalar.activation(negm2[:], m2r[:], AFT.Copy, scale=-1.0 / ZD)
                    tmp = mp.tile([128, ZD], F32, name=f"tmpa{s}")
                    nc.vector.scalar_tensor_tensor(tmp[:], xh[:], negm2[:], dxh[:], op0=AOP.mult, op1=AOP.add)
                    nc.vector.tensor_scalar_add(tmp[:], tmp[:], negm1[:])
                    da = mp.tile([128, ZD], F32, name=f"da{s}")
                    nc.vector.tensor_scalar_mul(da[:], tmp[:], inv[:])
                    return da

                def grad_matmul(da, wT_tiles, tag, width):
                    daT = []
                    for k in range(2):
                        t = mp.tile([128, 128], F32R, tag="daT" + str(k), bufs=2, name="daT")
                        pe_T(t[:], da[:, k * 128 : (k + 1) * 128])
                        daT.append(t)
                    outs = []
                    nchunk = (width + 511) // 512
                    for ch in range(nchunk):
                        w = min(512, width - ch * 512)
                        ps = pp.tile([128, 512], F32, tag="zpB", bufs=2, name="gm")
                        for k in range(2):
                            nc.tensor.matmul(
                                ps[:, :w],
                                daT[k][:],
                                wT_tiles[k][:, ch * 512 : ch * 512 + w],
                                start=(k == 0),
                                stop=(k == 1),
                            )
                        outs.append(ps)
                    return outs

                if KSTOP > 22:
                  da3 = bwd_layer(dh3, h3, sg3, xh3, inv3, 3)
                  w3T = transpose_w(wnf["w3"], "w3", ZD)
                  dh2ps = grad_matmul(da3, w3T, "dh2", ZD)[0]
                  dh2 = mp.tile([128, ZD], F32, name="dh2")
                  nc.vector.tensor_copy(dh2[:], dh2ps[:, :ZD])

                  da2 = bwd_layer(dh2, h2, sg2, xh2, inv2, 2)
                  w2T = transpose_w(wnf["w2"], "w2", ZD)
                  dh1ps = grad_matmul(da2, w2T, "dh1", ZD)[0]
                  dh1 = mp.tile([128, ZD], F32, name="dh1")
                  nc.vector.tensor_copy(dh1[:], dh1ps[:, :ZD])

                  da1 = bwd_layer(dh1, h1, sg1, xh1, inv1, 1)
                  w1T = transpose_w(w1f, "w1", ZD)  # [2][128, 1024]
                  dzps = grad_matmul(da1, w1T, "dz", HID)
                  dz_sb = mp.tile([128, HID], F32, name="dz_sb")
                  for ch, ps in enumerate(dzps):
                    nc.vector.tensor_copy(dz_sb[:, ch * 512 : (ch + 1) * 512], ps[:])
                  nc.sync.dma_start(dz_in_b[:], dz_sb[:])
                  if PH == 2:
                    nc.sync.dma_start(dx2_out[0:128, :], dz_sb[:, 0:DIM])

            # ---------------- phase: AllGather dz ----------------
            if PH >= 3:
              if KTIME:
                for i in range(NCORES):
                    nc.sync.dma_start(dz_ag_b[i * 128 : (i + 1) * 128, :], dz_in_b[:])
              else:
                nc.gpsimd.collective_compute(
                  "AllGather", AOP.bypass, replica_groups=rg,
                  ins=[dz_in_b[:].opt()], outs=[dz_ag_b[:].opt()],
                )

              # ---------------- phase F: load + transpose dz ----------------
              with (
                tc.tile_pool(name="bwd", bufs=1) as bp,
                tc.tile_pool(name="bwds", bufs=1) as bs,
                tc.tile_pool(name="bps", bufs=1, space="PSUM") as bps,
            ):
                dzT = []
                for wkk in range(8):
                    dzT.append(bp.tile([128, N], F32R, name=f"dzT{wkk}"))
                ti2 = 0
                for nt in range(NT):
                    dzrow = bs.tile([128, HID], F32, tag="dzrow", bufs=2, name="dzrow")
                    nc.sync.dma_start(dzrow[:], dz_ag_b[nt * 128 : (nt + 1) * 128, :])
                    for wkk in range(8):
                        tp = pp.tile([128, 128], F32, tag="tp", bufs=4, name="btp")
                        nc.tensor.transpose(tp[:], dzrow[:, wkk * 128 : (wkk + 1) * 128], ident[:])
                        evac(dzT[wkk][:, nt * 128 : (nt + 1) * 128], tp[:], ti2)
                        ti2 += 1

                # ---------------- phase G: backward TP + input grads ----------------
                for ch in range(NCH):
                    wb = []
                    for j in range(3):
                        t = bs.tile([128, HID], F32, tag=f"wb{j}", bufs=2, name="wb")
                        kg = ch * 3 + j
                        nc.sync.dma_start(t[:], w_in[kg * 128 : (kg + 1) * 128, :])
                        wb.append(t)
                    WT = []
                    for wkk in range(8):
                        WT.append(bs.tile([128, 384], F32R, tag=f"WT{wkk}", bufs=2, name="WT"))
                    for j in range(3):
                        for wkk in range(8):
                            tp = pp.tile([128, 128], F32, tag="tp", bufs=4, name="btp2")
                            nc.tensor.transpose(
                                tp[:], wb[j][:, wkk * 128 : (wkk + 1) * 128], ident[:]
                            )
                            evac(WT[wkk][:, j * 128 : (j + 1) * 128], tp[:], ti2)
                            ti2 += 1
                    for nt in range(NT):
                        gps = pp.tile([128, 384], F32, tag="zpA", bufs=2, name="gps")
                        for wkk in range(8):
                            nc.tensor.matmul(
                                gps[:],
                                dzT[wkk][:, nt * 128 : (nt + 1) * 128],
                                WT[wkk][:],
                                start=(wkk == 0),
                                stop=(wkk == 7),
                            )
                        g_blk = bs.tile([128, 384], F32, tag="g_blk", bufs=3, name="g_blk")
                        nc.scalar.activation(g_blk[:], gps[:], AFT.Copy)
                        if ch < 5:
                            # all three kgs are path0: batched term1
                            # junk3[p, j, v] = G[p, (u_j, v)] * x0[p, v]
                            junk3 = bs.tile([128, 384], F32, tag="junk3", bufs=3, name="junk3")
                            j3 = junk3[:].rearrange("p (j v) -> p j v", j=3)
                            g3 = g_blk[:].rearrange("p (j v) -> p j v", j=3)
                            xb = xt[nt][:, 0:128].unsqueeze(1).broadcast_to([128, 3, 128])
                            nc.gpsimd.tensor_mul(j3, g3, xb)
                            nc.vector.tensor_reduce(
                                dxut[nt][:, 3 * ch : 3 * ch + 3], j3, AXT.X, AOP.add
                            )
                            for j in range(3):
                                kg = ch * 3 + j
                                uc = xut[nt][:, kg : kg + 1]
                                dv = dx2t[nt][:, 0:128]
                                gv = g_blk[:, j * 128 : (j + 1) * 128]
                                nc.vector.scalar_tensor_tensor(
                                    dv, gv, uc, dv, op0=AOP.mult, op1=AOP.add
                                )
                            continue
                        for j in range(3):
                            kg = ch * 3 + j
                            path, groups = _kg_info(kg)
                            d = MULS[path][1]
                            c0j = j * 128
                            for (ul, mul, _d, c0, w) in groups:
                                gv = g_blk[:, c0j + c0 : c0j + c0 + w]
                                for m in range(d):
                                    col = _ucol_index(path, ul, m)
                                    uc = xut[nt][:, col : col + 1]
                                    xv = _x_path_view(xt[nt][:], nt, path, m, mul)
                                    dv = _x_path_view(dx2t[nt][:], nt, path, m, mul)
                                    junk = bs.tile([128, 128], F32, tag="junk", bufs=4, name="junk")
                                    # term1: dxu[:, col] = sum_v G[:, (u,v)] * x[:, (v,m)]
                                    # (multiply on GPSIMD to unload DVE; reduce on DVE)
                                    nc.gpsimd.tensor_mul(junk[:, :w], gv, xv)
                                    nc.vector.tensor_reduce(
                                        dxut[nt][:, col : col + 1], junk[:, :w], AXT.X, AOP.add
                                    )
                                    # term2: dx2[:, (v,m)] += G[:, (u,v)] * xu[:, col]
                                    nc.vector.scalar_tensor_tensor(
                                        dv, gv, uc, dv, op0=AOP.mult, op1=AOP.add
                                    )

              # ---------------- outputs ----------------
              for nt in range(NT):
                nc.sync.dma_start(dxu_out[nt * 128 : (nt + 1) * 128, :], dxut[nt][:])
                nc.sync.dma_start(dx2_out[nt * 128 : (nt + 1) * 128, :], dx2t[nt][:])

    nc.compile()
    return nc


_CACHED = None


def _get_kernel():
    global _CACHED
    if _CACHED is None:
        _CACHED = build_kernel()
    return _CACHED


def kernel(tensor_in, W0, W1, W2, w1, b1, g1, be1, w2, b2, g2, be2,
           w3, b3, g3, be3, w4, b4, **run_kwargs):
    nc = _get_kernel()
    tensor_in = np.ascontiguousarray(tensor_in, dtype=np.float32)
    vecs = np.ascontiguousarray(
        np.repeat(
            np.stack([b1, g1, be1, b2, g2, be2, b3, g3, be3, b4], 0).astype(np.float32),
            128,
            axis=0,
        )
    )
    w1 = np.ascontiguousarray(w1, dtype=np.float32)
    w2 = np.ascontiguousarray(w2, dtype=np.float32)
    w3 = np.ascontiguousarray(w3, dtype=np.float32)
    w4 = np.ascontiguousarray(w4, dtype=np.float32)
    in_maps = []
    ucols = []
    for c in range(NCORES):
        uc = host_ucols(c)
        ucols.append(uc)
        ws = host_wslice(np.asarray(W0), np.asarray(W1), np.asarray(W2), c)
        in_maps.append({
            "x_in": tensor_in,
            "xu_in": np.ascontiguousarray(tensor_in[:, uc]),
            "w_in": ws,
            "wr_in": ws,
            "w1_in": w1, "w1r_in": w1,
            "w2_in": w2, "w2r_in": w2,
            "w3_in": w3, "w3r_in": w3,
            "w4_in": w4, "w4r_in": w4,
            "vecs_in": vecs,
        })
    res = run_bass_kernel_spmd(nc, in_maps, core_ids=list(range(NCORES)), **run_kwargs)
    outs = res.results
    x_full = np.concatenate([outs[c]["xout"] for c in range(NCORES)], 0)
    y = np.zeros((N, DIM), dtype=np.float32)
    for c in range(NCORES):
        y += outs[c]["dx2_out"]
        np.add.at(y, (slice(None), ucols[c]), outs[c]["dxu_out"])
    if run_kwargs:
        return (x_full, y), res
    return (x_full, y)


# revision 43
# speedup vs baseline: 1.0298x; 1.0298x over previous
"""Trainium2 Bass kernel for nn_Equi_Nonlin_Grad_Module (e3nn FCTP l⊗l→0e + MLP, fwd + input-grad).

Strategy (8 cores): K-shard the 21504-row (u,v)-pair contraction dim of the
fused tensor-product weight Wcat across cores (by u-blocks within each irrep
path: 16/8/4 u's per core -> 2688 rows each).  Each core computes its s-slice
for all N on DVE, PE-transposes it, does the partial forward matmul in fp32r,
ReduceScatters z (rank c receives its 128-row n-shard), runs the MLP fwd+bwd
on its shard, AllGathers dz, computes G[:, slice] = dz @ Wslice^T (PE-transposed
W), and finally the per-node gradient contributions on DVE/GPSIMD.  Partial
input-gradients are summed on the host.
"""

import sys

for _p in ("/opt/trn_rl_repo",):
    if _p not in sys.path:
        sys.path.insert(0, _p)

import numpy as np
import concourse.bacc as bacc
import concourse.mybir as mybir
import concourse.tile as tile
from concourse import masks
from concourse.bass_utils import run_bass_kernel_spmd

F32 = mybir.dt.float32
F32R = mybir.dt.float32r
AOP = mybir.AluOpType
AFT = mybir.ActivationFunctionType
AXT = mybir.AxisListType

NCORES = 8
N = 1024
NT = 8  # n-tiles of 128
DIM = 480
HID = 1024
ZD = 256
MULS = ((128, 1), (64, 3), (32, 5))
PATH_OFF = (0, 128, 320)  # column offsets of each irrep path in x
ALPHA = 1.0 / np.sqrt(float(128 * 128 + 64 * 64 + 32 * 32))
# per-core u-counts per path
UC = (16, 8, 4)
NKG = 21  # 16 path0 + 4 path1 + 1 path2 pair-row tiles of 128
KROWS = NKG * 128  # 2688
NCH = 7  # backward chunks of 3 kg = 384 cols
EPS = 1e-6


def _kg_info(kg):
    """(path, list of (u_local, mul, d, s_col0, width)) for pair-tile kg."""
    if kg < 16:
        return 0, [(kg, 128, 1, 0, 128)]
    if kg < 20:
        j = kg - 16
        return 1, [(2 * j, 64, 3, 0, 64), (2 * j + 1, 64, 3, 64, 64)]
    return 2, [(q, 32, 5, q * 32, 32) for q in range(4)]


def _ucol_index(path, u_local, m):
    """Column in the per-core xu / dxu [*, 60] tensor."""
    if path == 0:
        return u_local
    if path == 1:
        return 16 + u_local * 3 + m
    return 40 + u_local * 5 + m


def host_ucols(core):
    """Global x-column index for each of the 60 per-core u-scalar columns."""
    cols = []
    for u in range(16 * core, 16 * core + 16):
        cols.append(0 + u)  # path0, d=1
    for u in range(8 * core, 8 * core + 8):
        for m in range(3):
            cols.append(128 + u * 3 + m)
    for u in range(4 * core, 4 * core + 4):
        for m in range(5):
            cols.append(320 + u * 5 + m)
    return np.array(cols, dtype=np.int64)


def host_wslice(W0, W1, W2, core):
    """Per-core [2688, 1024] slice of the pre-scaled fused weight."""
    w0 = W0.reshape(128 * 128, HID)
    w1 = W1.reshape(64 * 64, HID)
    w2 = W2.reshape(32 * 32, HID)
    rows = []
    for u in range(16 * core, 16 * core + 16):
        rows.append(w0[u * 128 : (u + 1) * 128] * (ALPHA / np.sqrt(1.0)))
    for j in range(4):
        for h in range(2):
            u = 8 * core + 2 * j + h
            rows.append(w1[u * 64 : (u + 1) * 64] * (ALPHA / np.sqrt(3.0)))
    for q in range(4):
        u = 4 * core + q
        rows.append(w2[u * 32 : (u + 1) * 32] * (ALPHA / np.sqrt(5.0)))
    return np.ascontiguousarray(np.concatenate(rows, 0), dtype=np.float32)


def _x_path_view(ap, nt_unused, path, m, mul):
    """[128, mul] strided view of the (v, m) columns of path in an x-tile AP."""
    off, d = PATH_OFF[path], MULS[path][1]
    if d == 1:
        return ap[:, off : off + mul]
    v = ap[:, off : off + mul * d].rearrange("p (v m) -> p m v", m=d)
    return v[:, m, :]


def build_kernel():
    import os

    PH = int(os.environ.get("KPHASE", "9"))  # 1=fwd-TP+RS, 2=+MLP, 3=+AG+bwd, 9=all
    KSTOP = int(os.environ.get("KSTOP", "99"))  # 20=L1, 21=fwd, 22=dh3, 99=all
    KTIME = int(os.environ.get("KTIME", "0"))  # 1: replace collectives with local DMA (for TimelineSim)
    nc = bacc.Bacc("TRN2", target_bir_lowering=False, debug=False, num_devices=NCORES)

    # ---------------- I/O ----------------
    x_in = nc.dram_tensor("x_in", [N, DIM], F32, kind="ExternalInput")
    xu_in = nc.dram_tensor("xu_in", [N, 60], F32, kind="ExternalInput")
    w_in = nc.dram_tensor("w_in", [KROWS, HID], F32, kind="ExternalInput")
    # fp32r alias of the same weight bytes: lets HWDGE load f32r tiles with no cast
    wr_in = nc.dram_tensor("wr_in", [KROWS, HID], F32R, kind="ExternalInput")
    w1_in = nc.dram_tensor("w1_in", [HID, ZD], F32, kind="ExternalInput")
    w2_in = nc.dram_tensor("w2_in", [ZD, ZD], F32, kind="ExternalInput")
    w3_in = nc.dram_tensor("w3_in", [ZD, ZD], F32, kind="ExternalInput")
    w4_in = nc.dram_tensor("w4_in", [ZD, ZD], F32, kind="ExternalInput")
    w1r_in = nc.dram_tensor("w1r_in", [HID, ZD], F32R, kind="ExternalInput")
    w2r_in = nc.dram_tensor("w2r_in", [ZD, ZD], F32R, kind="ExternalInput")
    w3r_in = nc.dram_tensor("w3r_in", [ZD, ZD], F32R, kind="ExternalInput")
    w4r_in = nc.dram_tensor("w4r_in", [ZD, ZD], F32R, kind="ExternalInput")
    vecs_in = nc.dram_tensor("vecs_in", [10 * 128, ZD], F32, kind="ExternalInput")
    xout = nc.dram_tensor("xout", [128, ZD], F32, kind="ExternalOutput")
    dxu_out = nc.dram_tensor("dxu_out", [N, 60], F32, kind="ExternalOutput")
    dx2_out = nc.dram_tensor("dx2_out", [N, DIM], F32, kind="ExternalOutput")

    rg = [list(range(NCORES))]
    VEC = {"b1": 0, "g1": 1, "be1": 2, "b2": 3, "g2": 4, "be2": 5, "b3": 6, "g3": 7, "be3": 8, "b4": 9}

    with tile.TileContext(nc) as tc:
        with (
            tc.tile_pool(name="const", bufs=1) as cp,
            tc.tile_pool(name="psum", bufs=1, space="PSUM") as pp,
            tc.tile_pool(name="dram", bufs=1, space="DRAM") as dram,
            tc.tile_pool(name="work", bufs=1) as wk,
        ):
            # ---------------- phase A: constants + persistent loads ----------------
            ident = cp.tile([128, 128], F32, name="ident")
            masks.make_identity(nc, ident[:])
            ones_f = cp.tile([128, 128], F32, name="ones_f")
            nc.gpsimd.memset(ones_f[:], 1.0)
            ones_r = cp.tile([128, 128], F32R, name="ones_r")
            nc.vector.tensor_copy(ones_r[:], ones_f[:])
            eps_t = cp.tile([128, 1], F32, name="eps_t")
            nc.gpsimd.memset(eps_t[:], EPS)

            xt = []
            xut = []
            dx2t = []
            dxut = []
            for nt in range(NT):
                t = cp.tile([128, DIM], F32, name=f"xt{nt}")
                nc.sync.dma_start(t[:], x_in[nt * 128 : (nt + 1) * 128, :])
                xt.append(t)
                t = cp.tile([128, 60], F32, name=f"xut{nt}")
                nc.sync.dma_start(t[:], xu_in[nt * 128 : (nt + 1) * 128, :])
                xut.append(t)
                t = cp.tile([128, DIM], F32, name=f"dx2t{nt}")
                nc.gpsimd.memset(t[:], 0.0)
                dx2t.append(t)
                t = cp.tile([128, 60], F32, name=f"dxut{nt}")
                dxut.append(t)

            # LN/bias vectors arrive pre-broadcast from the host: [10*128, ZD]
            vb = {}
            for k, i in VEC.items():
                t = cp.tile([128, ZD], F32, name=f"vb_{k}")
                nc.sync.dma_start(t[:], vecs_in[i * 128 : (i + 1) * 128, :])
                vb[k] = t

            # MLP weights: natural fp32r (fwd) and fp32 (for transposing)
            w1t, w1f = [], []
            for k in range(8):
                t = wk.tile([128, ZD], F32R, name=f"w1t{k}")
                nc.sync.dma_start(t[:], w1r_in[k * 128 : (k + 1) * 128, :])
                w1t.append(t)
                t = wk.tile([128, ZD], F32, name=f"w1f{k}")
                nc.sync.dma_start(t[:], w1_in[k * 128 : (k + 1) * 128, :])
                w1f.append(t)
            wnt = {}
            wnf = {}
            for nm, dtr, dt_ in (("w2", w2r_in, w2_in), ("w3", w3r_in, w3_in), ("w4", w4r_in, w4_in)):
                wnt[nm], wnf[nm] = [], []
                for k in range(2):
                    t = wk.tile([128, ZD], F32R, name=f"{nm}t{k}")
                    nc.sync.dma_start(t[:], dtr[k * 128 : (k + 1) * 128, :])
                    wnt[nm].append(t)
                    t = wk.tile([128, ZD], F32, name=f"{nm}f{k}")
                    nc.sync.dma_start(t[:], dt_[k * 128 : (k + 1) * 128, :])
                    wnf[nm].append(t)

            # DRAM bounce buffers for collectives
            z_part_b = dram.tile([N, HID], F32, name="z_part_b")
            z_rs_b = dram.tile([128, HID], F32, name="z_rs_b")
            dz_in_b = dram.tile([128, HID], F32, name="dz_in_b")
            dz_ag_b = dram.tile(
                [N, HID], F32, name="dz_ag_b", addr_space=("Local" if KTIME else "Shared")
            )

            def evac(dst, src, idx):
                # PSUM->SBUF evacuation of small transpose tiles: keep on ACT
                # (DVE is the bottleneck engine; ACT has headroom)
                if idx % 4 == 3:
                    nc.vector.tensor_copy(dst, src)
                else:
                    nc.scalar.activation(dst, src, AFT.Copy)

            # ---------------- phase B: forward tensor-product ----------------
            with (
                tc.tile_pool(name="wpool", bufs=1) as wp,
                tc.tile_pool(name="fwd", bufs=3) as fp,
            ):
                wt = []
                for kg in range(NKG):
                    t = wp.tile([128, HID], F32R, name=f"wt{kg}")
                    nc.sync.dma_start(t[:], wr_in[kg * 128 : (kg + 1) * 128, :])
                    wt.append(t)

                ei = 0
                for nt in range(NT):
                    zp0 = pp.tile([128, 512], F32, tag="zpA", bufs=2, name="zp0")
                    zp1 = pp.tile([128, 512], F32, tag="zpB", bufs=2, name="zp1")
                    # 1) build all 21 transposed s tiles for this n-tile
                    sTs = []
                    for kg in range(NKG):
                        path, groups = _kg_info(kg)
                        d = MULS[path][1]
                        s_blk = fp.tile([128, 128], F32, tag="s_blk", bufs=4, name="s_blk")
                        eng = nc.vector
                        for (ul, mul, _d, c0, w) in groups:
                            sv = s_blk[:, c0 : c0 + w]
                            for m in range(d):
                                uc = xut[nt][:, _ucol_index(path, ul, m) : _ucol_index(path, ul, m) + 1]
                                xv = _x_path_view(xt[nt][:], nt, path, m, mul)
                                if m == 0:
                                    # tensor_scalar_mul on ACT (Copy w/ per-partition scale)
                                    nc.scalar.activation(sv, xv, AFT.Copy, scale=uc)
                                else:
                                    eng.scalar_tensor_tensor(
                                        sv, xv, uc, sv, op0=AOP.mult, op1=AOP.add
                                    )
                        tp = pp.tile([128, 128], F32, tag="tp", bufs=4, name="tp")
                        nc.tensor.transpose(tp[:], s_blk[:], ident[:])
                        sT = fp.tile([128, 128], F32R, tag=f"sT{kg}", bufs=2, name="sT")
                        evac(sT[:], tp[:], ei)
                        ei += 1
                        sTs.append(sT)
                    # 2) dense back-to-back matmul accumulation
                    for kg in range(NKG):
                        nc.tensor.matmul(
                            zp0[:], sTs[kg][:], wt[kg][:, 0:512], start=(kg == 0), stop=(kg == NKG - 1)
                        )
                        nc.tensor.matmul(
                            zp1[:], sTs[kg][:], wt[kg][:, 512:HID], start=(kg == 0), stop=(kg == NKG - 1)
                        )
                    z_sb = fp.tile([128, HID], F32, tag="z_sb", bufs=2, name="z_sb")
                    nc.vector.tensor_copy(z_sb[:, 0:512], zp0[:])
                    nc.vector.tensor_copy(z_sb[:, 512:HID], zp1[:])
                    nc.sync.dma_start(z_part_b[nt * 128 : (nt + 1) * 128, :], z_sb[:])

            # ---------------- phase C: ReduceScatter z ----------------
            if KTIME:
                nc.sync.dma_start(z_rs_b[:], z_part_b[0:128, :])
            else:
                nc.gpsimd.collective_compute(
                    "ReduceScatter", AOP.add, replica_groups=rg,
                    ins=[z_part_b[:].opt()], outs=[z_rs_b[:].opt()],
                )

            if PH == 1:
                zdbg = wk.tile([128, HID], F32, name="zdbg")
                nc.sync.dma_start(zdbg[:], z_rs_b[:])
                nc.sync.dma_start(xout[:], zdbg[:, 0:ZD])

            # ---------------- phase D/E: MLP fwd + bwd on the local n-shard ----------------
            if PH >= 2:
              with (
                tc.tile_pool(name="mlp", bufs=1) as mp,
                tc.tile_pool(name="mps", bufs=1, space="PSUM") as mps,
            ):
                z_loc = mp.tile([128, HID], F32, name="z_loc")
                nc.sync.dma_start(z_loc[:], z_rs_b[:])

                ti = 0

                def pe_T(dst_f32r_ap, src_f32_ap):
                    nonlocal ti
                    tp = pp.tile([128, 128], F32, tag="tp", bufs=4, name="mtp")
                    nc.tensor.transpose(tp[:], src_f32_ap, ident[:])
                    evac(dst_f32r_ap, tp[:], ti)
                    ti += 1

                zT = []
                for k in range(8):
                    t = mp.tile([128, 128], F32R, name=f"zT{k}")
                    pe_T(t[:], z_loc[:, k * 128 : (k + 1) * 128])
                    zT.append(t)

                def ln_silu(a_ps, lidx, apply_silu=True):
                    """Input: psum [128, ZD] pre-bias. Returns (h, xh, inv, t_ln)."""
                    s = str(lidx)
                    a = mp.tile([128, ZD], F32, name=f"a{s}")
                    nc.vector.tensor_add(a[:], a_ps[:], vb["b" + s][:])
                    if KSTOP == 10 and lidx == 1:
                        nc.sync.dma_start(xout[:], a[:])
                        return a, a, a, a
                    red = mp.tile([128, 1], F32, name=f"red{s}")
                    nc.vector.tensor_reduce(red[:], a[:], AXT.X, AOP.add)
                    negmean = mp.tile([128, 1], F32, name=f"negmean{s}")
                    nc.scalar.activation(negmean[:], red[:], AFT.Copy, scale=-1.0 / ZD)
                    xc = mp.tile([128, ZD], F32, name=f"xc{s}")
                    nc.vector.tensor_scalar_add(xc[:], a[:], negmean[:])
                    if KSTOP == 11 and lidx == 1:
                        nc.sync.dma_start(xout[:], xc[:])
                        return a, a, a, a
                    junk = mp.tile([128, ZD], F32, tag="mjunk", bufs=2, name="mjunk")
                    vsum = mp.tile([128, 1], F32, name=f"vsum{s}")
                    nc.vector.tensor_mul(junk[:], xc[:], xc[:])
                    nc.vector.tensor_reduce(vsum[:], junk[:], AXT.X, AOP.add)
                    if KSTOP == 13 and lidx == 1:
                        nc.sync.dma_start(xout[:], junk[:])
                        return a, a, a, a
                    sig = mp.tile([128, 1], F32, name=f"sig{s}")
                    nc.scalar.activation(sig[:], vsum[:], AFT.Sqrt, scale=1.0 / ZD, bias=eps_t[:])
                    inv = mp.tile([128, 1], F32, name=f"inv{s}")
                    if KSTOP == 14 and lidx == 1:
                        nc.sync.dma_start(xout[:], junk[:])
                        return a, a, a, a
                    nc.vector.reciprocal(inv[:], sig[:])
                    if KSTOP == 12 and lidx == 1:
                        nc.sync.dma_start(xout[:], junk[:])
                        return a, a, a, a
                    xh = mp.tile([128, ZD], F32, name=f"xh{s}")
                    nc.vector.tensor_scalar_mul(xh[:], xc[:], inv[:])
                    t_ln = mp.tile([128, ZD], F32, name=f"tln{s}")
                    nc.vector.tensor_mul(t_ln[:], xh[:], vb["g" + s][:])
                    nc.vector.tensor_add(t_ln[:], t_ln[:], vb["be" + s][:])
                    sg = mp.tile([128, ZD], F32, name=f"sg{s}")
                    nc.scalar.activation(sg[:], t_ln[:], AFT.Sigmoid)
                    h = mp.tile([128, ZD], F32, name=f"h{s}")
                    nc.vector.tensor_mul(h[:], t_ln[:], sg[:])
                    return h, xh, inv, sg

                # layer 1
                ap1 = pp.tile([128, ZD], F32, tag="zpA", bufs=2, name="ap1")
                for k in range(8):
                    nc.tensor.matmul(ap1[:], zT[k][:], w1t[k][:], start=(k == 0), stop=(k == 7))
                h1, xh1, inv1, sg1 = ln_silu(ap1, 1)
                if KSTOP == 20:
                    nc.sync.dma_start(xout[:], h1[:])

                def fwd_layer(h_prev, wts, lidx):
                    hT = []
                    for k in range(2):
                        t = mp.tile([128, 128], F32R, name=f"hT{lidx}_{k}")
                        pe_T(t[:], h_prev[:, k * 128 : (k + 1) * 128])
                        hT.append(t)
                    aps = pp.tile([128, ZD], F32, tag="zpA", bufs=2, name=f"aps{lidx}")
                    for k in range(2):
                        nc.tensor.matmul(aps[:], hT[k][:], wts[k][:], start=(k == 0), stop=(k == 1))
                    return aps

                if KSTOP > 20:
                  ap2 = fwd_layer(h1, wnt["w2"], 2)
                  h2, xh2, inv2, sg2 = ln_silu(ap2, 2)
                  ap3 = fwd_layer(h2, wnt["w3"], 3)
                  h3, xh3, inv3, sg3 = ln_silu(ap3, 3)
                  ap4 = fwd_layer(h3, wnt["w4"], 4)
                  xo = mp.tile([128, ZD], F32, name="xo")
                  nc.vector.tensor_add(xo[:], ap4[:], vb["b4"][:])
                  nc.sync.dma_start(xout[:], xo[:])

                # ---- MLP backward ----
                def transpose_w(wf_tiles, nm, wide):
                    """build w^T fp32r tiles: wide=ZD cols -> [2][128, 128*len(wf)]"""
                    ncols = 128 * len(wf_tiles)
                    out = []
                    for k in range(wide // 128):
                        t = mp.tile([128, ncols], F32R, name=f"{nm}T{k}")
                        out.append(t)
                    for jt, wf in enumerate(wf_tiles):
                        for k in range(wide // 128):
                            pe_T(out[k][:, jt * 128 : (jt + 1) * 128], wf[:, k * 128 : (k + 1) * 128])
                    return out

                if KSTOP > 21:
                  w4T = transpose_w(wnf["w4"], "w4", ZD)
                  dh3ps = pp.tile([128, ZD], F32, tag="zpB", bufs=2, name="dh3ps")
                  for k in range(2):
                    nc.tensor.matmul(dh3ps[:], ones_r[:], w4T[k][:], start=(k == 0), stop=(k == 1))
                  dh3 = mp.tile([128, ZD], F32, name="dh3")
                  nc.vector.tensor_copy(dh3[:], dh3ps[:])

                def bwd_layer(dh, h, sg, xh, inv, lidx):
                    """returns da = dL/d(pre-LN activations) [128, ZD] fp32 sbuf"""
                    s = str(lidx)
                    # silu'(t) = sg + h*(1 - sg)  (h = t*sg)
                    ds = mp.tile([128, ZD], F32, tag="mjunk2", bufs=2, name="ds")
                    nc.vector.tensor_mul(ds[:], h[:], sg[:])
                    nc.vector.tensor_sub(ds[:], h[:], ds[:])
                    nc.vector.tensor_add(ds[:], ds[:], sg[:])
                    dt = mp.tile([128, ZD], F32, name=f"dt{s}")
                    nc.vector.tensor_mul(dt[:], dh[:], ds[:])
                    dxh = mp.tile([128, ZD], F32, name=f"dxh{s}")
                    nc.vector.tensor_mul(dxh[:], dt[:], vb["g" + s][:])
                    m1r = mp.tile([128, 1], F32, name=f"m1r{s}")
                    nc.vector.tensor_reduce(m1r[:], dxh[:], AXT.X, AOP.add)
                    negm1 = mp.tile([128, 1], F32, name=f"negm1{s}")
                    nc.scalar.activation(negm1[:], m1r[:], AFT.Copy, scale=-1.0 / ZD)
                    junk = mp.tile([128, ZD], F32, tag="mjunk", bufs=2, name="mjunkb")
                    m2r = mp.tile([128, 1], F32, name=f"m2r{s}")
                    nc.vector.tensor_mul(junk[:], dxh[:], xh[:])
                    nc.vector.tensor_reduce(m2r[:], junk[:], AXT.X, AOP.add)
                    negm2 = mp.tile([128, 1], F32, name=f"negm2{s}")
                    nc.scalar.activation(negm2[:], m2r[:], AFT.Copy, scale=-1.0 / ZD)
                    tmp = mp.tile([128, ZD], F32, name=f"tmpa{s}")
                    nc.vector.scalar_tensor_tensor(tmp[:], xh[:], negm2[:], dxh[:], op0=AOP.mult, op1=AOP.add)
                    nc.vector.tensor_scalar_add(tmp[:], tmp[:], negm1[:])
                    da = mp.tile([128, ZD], F32, name=f"da{s}")
                    nc.vector.tensor_scalar_mul(da[:], tmp[:], inv[:])
                    return da

                def grad_matmul(da, wT_tiles, tag, width):
                    daT = []
                    for k in range(2):
                        t = mp.tile([128, 128], F32R, tag="daT" + str(k), bufs=2, name="daT")
                        pe_T(t[:], da[:, k * 128 : (k + 1) * 128])
                        daT.append(t)
                    outs = []
                    nchunk = (width + 511) // 512
                    for ch in range(nchunk):
                        w = min(512, width - ch * 512)
                        ps = pp.tile([128, 512], F32, tag="zpB", bufs=2, name="gm")
                        for k in range(2):
                            nc.tensor.matmul(
                                ps[:, :w],
                                daT[k][:],
                                wT_tiles[k][:, ch * 512 : ch * 512 + w],
                                start=(k == 0),
                                stop=(k == 1),
                            )
                        outs.append(ps)
                    return outs

                if KSTOP > 22:
                  da3 = bwd_layer(dh3, h3, sg3, xh3, inv3, 3)
                  w3T = transpose_w(wnf["w3"], "w3", ZD)
                  dh2ps = grad_matmul(da3, w3T, "dh2", ZD)[0]
                  dh2 = mp.tile([128, ZD], F32, name="dh2")
                  nc.vector.tensor_copy(dh2[:], dh2ps[:, :ZD])

                  da2 = bwd_layer(dh2, h2, sg2, xh2, inv2, 2)
                  w2T = transpose_w(wnf["w2"], "w2", ZD)
                  dh1ps = grad_matmul(da2, w2T, "dh1", ZD)[0]
                  dh1 = mp.tile([128, ZD], F32, name="dh1")
                  nc.vector.tensor_copy(dh1[:], dh1ps[:, :ZD])

                  da1 = bwd_layer(dh1, h1, sg1, xh1, inv1, 1)
                  w1T = transpose_w(w1f, "w1", ZD)  # [2][128, 1024]
                  dzps = grad_matmul(da1, w1T, "dz", HID)
                  dz_sb = mp.tile([128, HID], F32, name="dz_sb")
                  for ch, ps in enumerate(dzps):
                    nc.vector.tensor_copy(dz_sb[:, ch * 512 : (ch + 1) * 512], ps[:])
                  nc.sync.dma_start(dz_in_b[:], dz_sb[:])
                  if PH == 2:
                    nc.sync.dma_start(dx2_out[0:128, :], dz_sb[:, 0:DIM])

            # ---------------- phase: AllGather dz ----------------
            if PH >= 3:
              if KTIME:
                for i in range(NCORES):
                    nc.sync.dma_start(dz_ag_b[i * 128 : (i + 1) * 128, :], dz_in_b[:])
              else:
                nc.gpsimd.collective_compute(
                  "AllGather", AOP.bypass, replica_groups=rg,
                  ins=[dz_in_b[:].opt()], outs=[dz_ag_b[:].opt()],
                )

              # ---------------- phase F: load + transpose dz ----------------
              with (
                tc.tile_pool(name="bwd", bufs=1) as bp,
                tc.tile_pool(name="bwds", bufs=1) as bs,
                tc.tile_pool(name="bps", bufs=1, space="PSUM") as bps,
            ):
                dzT = []
                for wkk in range(8):
                    dzT.append(bp.tile([128, N], F32R, name=f"dzT{wkk}"))
                ti2 = 0
                for nt in range(NT):
                    dzrow = bs.tile([128, HID], F32, tag="dzrow", bufs=2, name="dzrow")
                    nc.sync.dma_start(dzrow[:], dz_ag_b[nt * 128 : (nt + 1) * 128, :])
                    for wkk in range(8):
                        tp = pp.tile([128, 128], F32, tag="tp", bufs=4, name="btp")
                        nc.tensor.transpose(tp[:], dzrow[:, wkk * 128 : (wkk + 1) * 128], ident[:])
                        evac(dzT[wkk][:, nt * 128 : (nt + 1) * 128], tp[:], ti2)
                        ti2 += 1

                # ---------------- phase G: backward TP + input grads ----------------
                for ch in range(NCH):
                    wb = []
                    for j in range(3):
                        t = bs.tile([128, HID], F32, tag=f"wb{j}", bufs=2, name="wb")
                        kg = ch * 3 + j
                        nc.sync.dma_start(t[:], w_in[kg * 128 : (kg + 1) * 128, :])
                        wb.append(t)
                    WT = []
                    for wkk in range(8):
                        WT.append(bs.tile([128, 384], F32R, tag=f"WT{wkk}", bufs=2, name="WT"))
                    for j in range(3):
                        for wkk in range(8):
                            tp = pp.tile([128, 128], F32, tag="tp", bufs=4, name="btp2")
                            nc.tensor.transpose(
                                tp[:], wb[j][:, wkk * 128 : (wkk + 1) * 128], ident[:]
                            )
                            evac(WT[wkk][:, j * 128 : (j + 1) * 128], tp[:], ti2)
                            ti2 += 1
                    for nt in range(NT):
                        gps = pp.tile([128, 384], F32, tag="zpA", bufs=2, name="gps")
                        for wkk in range(8):
                            nc.tensor.matmul(
                                gps[:],
                                dzT[wkk][:, nt * 128 : (nt + 1) * 128],
                                WT[wkk][:],
                                start=(wkk == 0),
                                stop=(wkk == 7),
                            )
                        g_blk = bs.tile([128, 384], F32, tag="g_blk", bufs=8, name="g_blk")
                        nc.scalar.activation(g_blk[:], gps[:], AFT.Copy)
                        if ch < 5:
                            # all three kgs are path0: batched term1
                            # junk3[p, j, v] = G[p, (u_j, v)] * x0[p, v]
                            junk3 = bs.tile([128, 384], F32, tag="junk3", bufs=4, name="junk3")
                            j3 = junk3[:].rearrange("p (j v) -> p j v", j=3)
                            g3 = g_blk[:].rearrange("p (j v) -> p j v", j=3)
                            xb = xt[nt][:, 0:128].unsqueeze(1).broadcast_to([128, 3, 128])
                            nc.gpsimd.tensor_mul(j3, g3, xb)
                            nc.vector.tensor_reduce(
                                dxut[nt][:, 3 * ch : 3 * ch + 3], j3, AXT.X, AOP.add
                            )
                            for j in range(3):
                                kg = ch * 3 + j
                                uc = xut[nt][:, kg : kg + 1]
                                dv = dx2t[nt][:, 0:128]
                                gv = g_blk[:, j * 128 : (j + 1) * 128]
                                nc.vector.scalar_tensor_tensor(
                                    dv, gv, uc, dv, op0=AOP.mult, op1=AOP.add
                                )
                            continue
                        for j in range(3):
                            kg = ch * 3 + j
                            path, groups = _kg_info(kg)
                            d = MULS[path][1]
                            c0j = j * 128
                            nu = len(groups)
                            mul = groups[0][1]
                            if path != 0:
                                # batched term1 over the nu u-blocks of this kg
                                u0 = groups[0][0]
                                for m in range(d):
                                    xv = _x_path_view(xt[nt][:], nt, path, m, mul)
                                    xb = xv.unsqueeze(1).broadcast_to([128, nu, mul])
                                    junk = bs.tile([128, 128], F32, tag="junk", bufs=6, name="junk")
                                    j3 = junk[:].rearrange("p (u v) -> p u v", u=nu)
                                    g3 = g_blk[:, c0j : c0j + 128].rearrange("p (u v) -> p u v", u=nu)
                                    nc.gpsimd.tensor_mul(j3, g3, xb)
                                    base = 16 if path == 1 else 40
                                    dxv = dxut[nt][:, base : base + (8 if path == 1 else 4) * d]
                                    dxv = dxv.rearrange("p (u m) -> p m u", m=d)[:, m, u0 : u0 + nu]
                                    nc.vector.tensor_reduce(dxv, j3, AXT.X, AOP.add)
                            for (ul, mulw, _d, c0, w) in groups:
                                gv = g_blk[:, c0j + c0 : c0j + c0 + w]
                                for m in range(d):
                                    col = _ucol_index(path, ul, m)
                                    uc = xut[nt][:, col : col + 1]
                                    xv = _x_path_view(xt[nt][:], nt, path, m, mulw)
                                    dv = _x_path_view(dx2t[nt][:], nt, path, m, mulw)
                                    if path == 0:
                                        junk = bs.tile([128, 128], F32, tag="junk", bufs=6, name="junk")
                                        # term1: dxu[:, col] = sum_v G[:, (u,v)] * x[:, (v,m)]
                                        nc.gpsimd.tensor_mul(junk[:, :w], gv, xv)
                                        nc.vector.tensor_reduce(
                                            dxut[nt][:, col : col + 1], junk[:, :w], AXT.X, AOP.add
                                        )
                                    # term2: dx2[:, (v,m)] += G[:, (u,v)] * xu[:, col]
                                    nc.vector.scalar_tensor_tensor(
                                        dv, gv, uc, dv, op0=AOP.mult, op1=AOP.add
                                    )

              # ---------------- outputs ----------------
              for nt in range(NT):
                nc.sync.dma_start(dxu_out[nt * 128 : (nt + 1) * 128, :], dxut[nt][:])
                nc.sync.dma_start(dx2_out[nt * 128 : (nt + 1) * 128, :], dx2t[nt][:])

    nc.compile()
    return nc


_CACHED = None


def _get_kernel():
    global _CACHED
    if _CACHED is None:
        _CACHED = build_kernel()
    return _CACHED


def kernel(tensor_in, W0, W1, W2, w1, b1, g1, be1, w2, b2, g2, be2,
           w3, b3, g3, be3, w4, b4, **run_kwargs):
    nc = _get_kernel()
    tensor_in = np.ascontiguousarray(tensor_in, dtype=np.float32)
    vecs = np.ascontiguousarray(
        np.repeat(
            np.stack([b1, g1, be1, b2, g2, be2, b3, g3, be3, b4], 0).astype(np.float32),
            128,
            axis=0,
        )
    )
    w1 = np.ascontiguousarray(w1, dtype=np.float32)
    w2 = np.ascontiguousarray(w2, dtype=np.float32)
    w3 = np.ascontiguousarray(w3, dtype=np.float32)
    w4 = np.ascontiguousarray(w4, dtype=np.float32)
    in_maps = []
    ucols = []
    for c in range(NCORES):
        uc = host_ucols(c)
        ucols.append(uc)
        ws = host_wslice(np.asarray(W0), np.asarray(W1), np.asarray(W2), c)
        in_maps.append({
            "x_in": tensor_in,
            "xu_in": np.ascontiguousarray(tensor_in[:, uc]),
            "w_in": ws,
            "wr_in": ws,
            "w1_in": w1, "w1r_in": w1,
            "w2_in": w2, "w2r_in": w2,
            "w3_in": w3, "w3r_in": w3,
            "w4_in": w4, "w4r_in": w4,
            "vecs_in": vecs,
        })
    res = run_bass_kernel_spmd(nc, in_maps, core_ids=list(range(NCORES)), **run_kwargs)
    outs = res.results
    x_full = np.concatenate([outs[c]["xout"] for c in range(NCORES)], 0)
    y = np.zeros((N, DIM), dtype=np.float32)
    for c in range(NCORES):
        y += outs[c]["dx2_out"]
        np.add.at(y, (slice(None), ucols[c]), outs[c]["dxu_out"])
    if run_kwargs:
        return (x_full, y), res
    return (x_full, y)
